# revision 27
# baseline (speedup 1.0000x reference)
"""Binarized ResNet Bottleneck block (sign-binarized convs + BN + residual)
for Trainium2, data-parallel over 8 NeuronCores (8 images per core).

Math (per reference):
  out1 = BN1(conv1x1(sign(x),  sign(w1)))        # 1024 -> 256
  out2 = BN2(conv3x3(sign(out1), sign(w2)))      # 256 -> 256, pad 1
  out3 = BN3(conv1x1(sign(out2), sign(w3)))      # 256 -> 1024
  y    = out3 + x
(The htanh's in the reference only feed sign(), and sign(htanh(t)) == sign(t),
so they are dropped. Binarized values are exactly +-1 (or 0) in bf16 and conv
accumulations are exact small integers in fp32 PSUM, so matmuls are exact.)

Layout strategy per core (8 images, processed in 4 groups of G=2):
  - activations live in SBUF as [128 chan-partitions, chan_tile, img, pixels]
  - conv = accumulated 128x128xN matmuls; conv2's 3x3 uses 9 shifted-window
    matmuls over a zero-padded 16x16 per-image spatial layout.
  - BN (x*scale+shift, separate f32 roundings on VectorE to match the
    reference's unfused mul+add) then Sign on ScalarE between layers.
  - residual add on VectorE, output DMA'd back in a host-friendly layout.
"""

import os
import sys

import numpy as np
import ml_dtypes

N_CORES = 8
B = 64              # global batch
CIN = 1024
P = 256             # bottleneck width
NPX = 196           # 14*14
G = 2               # images per group
NGRP = 4            # groups per core  (8 images / G)
BPC = B // N_CORES  # images per core

_EPS = 1e-5

_state = {}


def _build_nc():
    import concourse.bass as bass
    import concourse.mybir as mybir
    from concourse import bacc
    from concourse.tile import TileContext

    fp32 = mybir.dt.float32
    bf16 = mybir.dt.bfloat16
    f8 = mybir.dt.float8e4
    DR = mybir.MatmulPerfMode.DoubleRow
    SIGN = mybir.ActivationFunctionType.Sign
    COPY = mybir.ActivationFunctionType.Copy
    MULT = mybir.AluOpType.mult
    ADD = mybir.AluOpType.add

    # Bacc (not plain Bass): its compile() pass splits multi-sem waits into
    # EventSemaphore instructions (HW allows only 1 wait per instruction).
    nc = bacc.Bacc(None, target_bir_lowering=False)

    xt = nc.dram_tensor("xt", [NGRP, 128, 8, G, NPX], fp32, kind="ExternalInput")
    # all binarized fp8 weights in one tensor, DoubleRow-interleaved:
    # cols [0:2048]=w1 (4 kpair x 2 ko x 256), [2048:6656]=w2 (9 tap x 2 ko
    # x 256), [6656:8704]=w3 (2 ko x 1024)
    wb = nc.dram_tensor("wb", [128, 8704], f8, kind="ExternalInput")
    # BN params in one tensor: sc1(2) sh1(2) sc2(2) sh2(2) sc3(8) sh3(8)
    bnp = nc.dram_tensor("bnp", [128, 24], fp32, kind="ExternalInput")
    yt = nc.dram_tensor("yt", [NGRP, 128, 8, G, NPX], fp32, kind="ExternalOutput")

    with TileContext(nc) as tc:
        with (
            tc.tile_pool(name="consts", bufs=1) as cpool,
            tc.tile_pool(name="xin_pool", bufs=4) as xin_pool,
            tc.tile_pool(name="xb1_pool", bufs=4) as xb1_pool,
            tc.tile_pool(name="xb2_pool", bufs=2) as xb2_pool,
            tc.tile_pool(name="xb3_pool", bufs=2) as xb3_pool,
            tc.tile_pool(name="tmp_pool", bufs=4) as tmp_pool,
            tc.tile_pool(name="out_pool", bufs=2) as out_pool,
            tc.tile_pool(name="ps1_pool", bufs=2, space="PSUM") as ps1_pool,
            tc.tile_pool(name="ps2_pool", bufs=2, space="PSUM") as ps2_pool,
            tc.tile_pool(name="ps3_pool", bufs=2, space="PSUM") as ps3_pool,
        ):
            wb_sb = cpool.tile([128, 8704], f8, name="wb_sb")
            w1_sb = wb_sb[:, 0:2048].rearrange("p (t k c) -> p t k c", t=4, k=2)
            w2_sb = wb_sb[:, 2048:6656].rearrange(
                "p (t k c) -> p t k c", t=9, k=2
            )
            w3_sb = wb_sb[:, 6656:8704].rearrange("p (k c) -> p k c", k=2)

            bnp_sb = cpool.tile([128, 24], fp32, name="bnp_sb")
            nc.sync.dma_start(bnp_sb, bnp[:])
            sc1_sb = bnp_sb[:, 0:2]
            sh1_sb = bnp_sb[:, 2:4]
            sc2_sb = bnp_sb[:, 4:6]
            sh2_sb = bnp_sb[:, 6:8]
            sc3_sb = bnp_sb[:, 8:16]
            sh3_sb = bnp_sb[:, 16:24]

            # Observer ops: several ISA structs (TensorScalarPtr, Activation
            # with AP scale/bias) only fit ONE sync-wait command, so make
            # each compute engine observe the const DMAs once up front;
            # Tile's vector clock then subsumes those waits downstream.
            scr_a = cpool.tile([128, 24], fp32, name="scr_a")
            nc.scalar.activation(scr_a, bnp_sb, COPY)
            scr_v = cpool.tile([128, 24], fp32, name="scr_v")
            nc.vector.tensor_tensor(scr_v, bnp_sb, bnp_sb, MULT)
            nc.tensor.ldweights(wb_sb[:, 0:128])

            # persistent zero-padded conv2-input buffers (border stays 0;
            # only the 14x14 interior is rewritten each group)
            xb2_bufs = []
            for i in range(2):
                xb2_buf = cpool.tile([128, 2, G, 256], f8, name=f"xb2_{i}")
                nc.scalar.memzero(xb2_buf)
                xb2_bufs.append(xb2_buf)

            # ---- startup: load + binarize ALL inputs up front ------------
            # (ACT's queue is in-order; hoisting the sign-ins keeps later
            # group boundaries from stalling PE behind them. DMA issue
            # order prioritizes what the first matmuls need.)
            xins, xb1s = [], []
            for g in range(NGRP):
                xin = xin_pool.tile([128, 8, G, NPX], fp32, name=f"xin{g}", tag="xin")
                xins.append(xin)
                xb1 = xb1_pool.tile([128, 8, G, NPX], f8, name=f"xb1{g}", tag="xb1")
                xb1s.append(xb1)
            # first group in two halves so sign-in overlaps its own DMA.
            # Only groups 0/1 are sign-binarized up front: ACT's queue is
            # in-order, so sign-in(g) for later groups is emitted inside
            # group g-1 (1-group lookahead) to avoid head-of-line blocking.
            nc.sync.dma_start(wb_sb[:, 0:2048], wb[:, 0:2048])      # w1 first
            for q in range(4):
                nc.sync.dma_start(
                    xins[0][:, 2 * q:2 * q + 2], xt[0, :, 2 * q:2 * q + 2]
                )
            for q in range(4):
                nc.scalar.activation(
                    xb1s[0][:, 2 * q:2 * q + 2], xins[0][:, 2 * q:2 * q + 2],
                    SIGN,
                )
            nc.sync.dma_start(wb_sb[:, 2048:8704], wb[:, 2048:8704])
            for g in range(1, NGRP):
                nc.sync.dma_start(xins[g], xt[g])
            nc.scalar.activation(xb1s[1], xins[1], SIGN)

            for g in range(NGRP):
                xin = xins[g]
                xb1 = xb1s[g]
                xtch = tmp_pool.tile([128, G, 1], fp32, name="xtch", tag="xtch")
                nc.vector.tensor_tensor(
                    xtch, xin[:, 0, :, 0:1], xin[:, 0, :, 0:1], MULT
                )
                # xb2: conv2 input in zero-padded 16x16 spatial layout
                xb2 = xb2_bufs[g % 2]
                for m in range(2):
                    ps1 = ps1_pool.tile([128, G * NPX], fp32, name="ps1")
                    for t in range(4):
                        nc.tensor.matmul(
                            ps1,
                            w1_sb[:, t, :, m * 128:(m + 1) * 128],
                            xb1[:, 2 * t:2 * t + 2].rearrange(
                                "p k b n -> p k (b n)"
                            ),
                            start=(t == 0),
                            stop=(t == 3),
                            perf_mode=DR,
                        )
                    # BN1+sign, bit-exact two-rounding on ACT:
                    #   t = RN(psum*scale)  (Copy w/ per-partition scale)
                    #   xb2 = Sign(fma(t, 1.0, shift)) = Sign(RN(t+shift))
                    t1 = tmp_pool.tile([128, G, NPX], fp32, name="t1", tag="tmp12")
                    nc.scalar.activation(
                        t1,
                        ps1.rearrange("p (b n) -> p b n", b=G),
                        COPY,
                        scale=sc1_sb[:, m:m + 1],
                    )
                    dst = xb2[:, m].rearrange("p b (h w) -> p b h w", h=16)[
                        :, :, 1:15, 1:15
                    ]
                    nc.scalar.activation(
                        dst,
                        t1.rearrange("p b (h w) -> p b h w", h=14),
                        SIGN,
                        bias=sh1_sb[:, m:m + 1],
                    )

                # sign-in for group g+2 (lookahead; ACT is idle-ish here)
                if g + 2 < NGRP:
                    nc.scalar.activation(xb1s[g + 2], xins[g + 2], SIGN)

                # ---- conv2 (3x3 pad 1, 256->256) + BN2 + sign ------------
                xb3 = xb3_pool.tile([128, 2, G, NPX], f8, name="xb3")
                for m in range(2):
                    ps2 = ps2_pool.tile([128, G, 512], fp32, name="ps2")
                    for tap in range(9):
                        ky, kx = tap // 3, tap % 3
                        wsl = w2_sb[:, tap, :, m * 128:(m + 1) * 128]
                        for b in range(G):
                            xv = xb2[:, :, b].rearrange(
                                "p k (h w) -> p k h w", h=16
                            )
                            nc.tensor.matmul(
                                ps2[:, b, :NPX],
                                wsl,
                                xv[:, :, ky:ky + 14, kx:kx + 14],
                                start=(tap == 0),
                                stop=(tap == 8),
                                perf_mode=DR,
                                skip_group_check=True,
                            )
                    t2 = tmp_pool.tile([128, G, NPX], fp32, name="t2", tag="tmp12")
                    nc.scalar.activation(
                        t2,
                        ps2[:, :, :NPX],
                        COPY,
                        scale=sc2_sb[:, m:m + 1],
                    )
                    nc.scalar.activation(
                        xb3[:, m], t2, SIGN, bias=sh2_sb[:, m:m + 1]
                    )

                # ---- conv3 (1x1, 256->1024) + BN3 + residual -------------
                # m-tiles paired into one 2-bank psum tile so each DVE evac
                # op covers 784 elements (amortizes per-op overhead).
                out_sb = out_pool.tile([128, 8, G, NPX], fp32, name="out_sb")
                for mm in range(4):
                    ps3s = []
                    for j in range(2):
                        m = 2 * mm + j
                        ps3 = ps3_pool.tile([128, G * NPX], fp32, name="ps3",
                                            tag="ps3")
                        ps3s.append(ps3)
                        nc.tensor.matmul(
                            ps3,
                            w3_sb[:, :, m * 128:(m + 1) * 128],
                            xb3.rearrange("p k b n -> p k (b n)"),
                            start=True,
                            stop=True,
                            perf_mode=DR,
                        )
                    # BN3 (x*scale+shift, two roundings) + residual add.
                    # (TensorScalarPtr is fine now: Bacc's compile() spills
                    # excess sem waits onto EventSemaphore instructions.)
                    m0 = 2 * mm
                    t3 = tmp_pool.tile([128, 2, G, NPX], fp32, name="t3",
                                       tag="t3")
                    for j in range(2):
                        nc.vector.tensor_scalar(
                            t3[:, j],
                            ps3s[j].rearrange("p (b n) -> p b n", b=G),
                            sc3_sb[:, m0 + j:m0 + j + 1],
                            sh3_sb[:, m0 + j:m0 + j + 1],
                            MULT,
                            ADD,
                        )
                    nc.vector.tensor_add(
                        out_sb[:, m0:m0 + 2], t3, xin[:, m0:m0 + 2]
                    )
                    nc.sync.dma_start(
                        yt[g, :, m0:m0 + 2], out_sb[:, m0:m0 + 2]
                    )

    nc.compile()
    return nc


def _bn_params(g, b, m, v):
    """scale/shift computed with the same jax expressions as the reference."""
    import jax
    import jax.numpy as jnp
    from jax import lax

    ge, be, me, ve = (jnp.asarray(t) for t in (g, b, m, v))
    scale = ge * lax.rsqrt(ve + _EPS)
    shift = be - ge * me * lax.rsqrt(ve + _EPS)
    return np.asarray(scale, np.float32), np.asarray(shift, np.float32)


def _prep_inputs(inputs):
    """Host-side prep: shard batch, binarize weights, fold BN params."""
    f8 = ml_dtypes.float8_e4m3
    x = np.ascontiguousarray(np.asarray(inputs["x"], np.float32))

    # weights -> sign -> fp8e4 (exact for +-1), DoubleRow-interleaved
    # layouts: [128 ki, kpair, ko, cout] where channel = (2*t+ko)*128+ki
    w1 = np.sign(np.asarray(inputs["w1"], np.float32)[:, :, 0, 0])        # [256,1024]
    w1b = np.ascontiguousarray(
        w1.T.reshape(4, 2, 128, 256).transpose(2, 0, 1, 3).astype(f8)
    )                                                                      # [128,4,2,256]
    w2 = np.sign(np.asarray(inputs["w2"], np.float32))                     # [256,256,3,3]
    w2b = np.ascontiguousarray(
        w2.transpose(1, 2, 3, 0)                                           # [ci,ky,kx,co]
        .reshape(2, 128, 9, 256)                                           # [ko,ki,tap,co]
        .transpose(1, 2, 0, 3)
        .astype(f8)
    )                                                                      # [128,9,2,256]
    w3 = np.sign(np.asarray(inputs["w3"], np.float32)[:, :, 0, 0])         # [1024,256]
    w3b = np.ascontiguousarray(
        w3.T.reshape(2, 128, 1024).transpose(1, 0, 2).astype(f8)
    )                                                                      # [128,2,1024]

    sc1, sh1 = _bn_params(inputs["g1"], inputs["b1"], inputs["m1"], inputs["v1"])
    sc2, sh2 = _bn_params(inputs["g2"], inputs["b2"], inputs["m2"], inputs["v2"])
    sc3, sh3 = _bn_params(inputs["g3"], inputs["b3"], inputs["m3"], inputs["v3"])

    wb = np.concatenate(
        [w1b.reshape(128, -1), w2b.reshape(128, -1), w3b.reshape(128, -1)],
        axis=1,
    )
    bnp = np.concatenate(
        [
            sc1.reshape(2, 128).T, sh1.reshape(2, 128).T,
            sc2.reshape(2, 128).T, sh2.reshape(2, 128).T,
            sc3.reshape(8, 128).T, sh3.reshape(8, 128).T,
        ],
        axis=1,
    ).astype(np.float32)
    common = {
        "wb": np.ascontiguousarray(wb),
        "bnp": np.ascontiguousarray(bnp),
    }

    # x -> per-core [NGRP, 128, 8kt, G, 196]
    xr = x.reshape(N_CORES, NGRP, G, 8, 128, NPX)  # (core, grp, img, kt, p, n)
    in_maps = []
    for c in range(N_CORES):
        xt = np.ascontiguousarray(xr[c].transpose(0, 3, 2, 1, 4))
        in_maps.append({"xt": xt, **common})
    return in_maps


def _assemble_output(results):
    """results: list of per-core dicts with 'yt' [NGRP,128,8,G,196]."""
    y = np.empty((N_CORES, NGRP, G, 8, 128, NPX), np.float32)
    for c, r in enumerate(results):
        y[c] = np.asarray(r["yt"]).transpose(0, 3, 2, 1, 4)
    return np.ascontiguousarray(
        y.reshape(B, CIN, 14, 14)
    )


def _run(inputs, trace=False):
    from concourse.bass_utils import run_bass_kernel_spmd

    if "nc" not in _state:
        _state["nc"] = _build_nc()
    nc = _state["nc"]
    in_maps = _prep_inputs(inputs)
    res = run_bass_kernel_spmd(
        nc, in_maps, core_ids=list(range(N_CORES)), trace=trace
    )
    return _assemble_output(res.results), res


def kernel(**inputs):
    out, _ = _run(inputs, trace=False)
    return out


# revision 28
# speedup vs baseline: 1.0695x; 1.0695x over previous
"""Binarized ResNet Bottleneck block (sign-binarized convs + BN + residual)
for Trainium2, data-parallel over 8 NeuronCores (8 images per core).

Math (per reference):
  out1 = BN1(conv1x1(sign(x),  sign(w1)))        # 1024 -> 256
  out2 = BN2(conv3x3(sign(out1), sign(w2)))      # 256 -> 256, pad 1
  out3 = BN3(conv1x1(sign(out2), sign(w3)))      # 256 -> 1024
  y    = out3 + x
(The htanh's in the reference only feed sign(), and sign(htanh(t)) == sign(t),
so they are dropped. Binarized values are exactly +-1 (or 0) in bf16 and conv
accumulations are exact small integers in fp32 PSUM, so matmuls are exact.)

Layout strategy per core (8 images, processed in 4 groups of G=2):
  - activations live in SBUF as [128 chan-partitions, chan_tile, img, pixels]
  - conv = accumulated 128x128xN matmuls; conv2's 3x3 uses 9 shifted-window
    matmuls over a zero-padded 16x16 per-image spatial layout.
  - BN (x*scale+shift, separate f32 roundings on VectorE to match the
    reference's unfused mul+add) then Sign on ScalarE between layers.
  - residual add on VectorE, output DMA'd back in a host-friendly layout.
"""

import os
import sys

import numpy as np
import ml_dtypes

N_CORES = 8
B = 64              # global batch
CIN = 1024
P = 256             # bottleneck width
NPX = 196           # 14*14
G = 2               # images per group
NGRP = 4            # groups per core  (8 images / G)
BPC = B // N_CORES  # images per core

_EPS = 1e-5

_state = {}


def _build_nc():
    import concourse.bass as bass
    import concourse.mybir as mybir
    from concourse import bacc
    from concourse.tile import TileContext

    fp32 = mybir.dt.float32
    bf16 = mybir.dt.bfloat16
    f8 = mybir.dt.float8e4
    DR = mybir.MatmulPerfMode.DoubleRow
    SIGN = mybir.ActivationFunctionType.Sign
    COPY = mybir.ActivationFunctionType.Copy
    MULT = mybir.AluOpType.mult
    ADD = mybir.AluOpType.add

    # Bacc (not plain Bass): its compile() pass splits multi-sem waits into
    # EventSemaphore instructions (HW allows only 1 wait per instruction).
    nc = bacc.Bacc(None, target_bir_lowering=False)

    xt = nc.dram_tensor("xt", [NGRP, 128, 8, G, NPX], fp32, kind="ExternalInput")
    # all binarized fp8 weights in one tensor, DoubleRow-interleaved:
    # cols [0:2048]=w1 (4 kpair x 2 ko x 256), [2048:6656]=w2 (9 tap x 2 ko
    # x 256), [6656:8704]=w3 (2 ko x 1024)
    wb = nc.dram_tensor("wb", [128, 8704], f8, kind="ExternalInput")
    # BN params in one tensor: sc1(2) sh1(2) sc2(2) sh2(2) sc3(8) sh3(8)
    bnp = nc.dram_tensor("bnp", [128, 24], fp32, kind="ExternalInput")
    yt = nc.dram_tensor("yt", [NGRP, 128, 8, G, NPX], fp32, kind="ExternalOutput")

    with TileContext(nc) as tc:
        with (
            tc.tile_pool(name="consts", bufs=1) as cpool,
            tc.tile_pool(name="xin_pool", bufs=4) as xin_pool,
            tc.tile_pool(name="xb1_pool", bufs=4) as xb1_pool,
            tc.tile_pool(name="xb2_pool", bufs=2) as xb2_pool,
            tc.tile_pool(name="xb3_pool", bufs=2) as xb3_pool,
            tc.tile_pool(name="tmp_pool", bufs=4) as tmp_pool,
            tc.tile_pool(name="out_pool", bufs=2) as out_pool,
            tc.tile_pool(name="ps1_pool", bufs=2, space="PSUM") as ps1_pool,
            tc.tile_pool(name="ps2_pool", bufs=1, space="PSUM") as ps2_pool,
            tc.tile_pool(name="ps3_pool", bufs=4, space="PSUM") as ps3_pool,
        ):
            wb_sb = cpool.tile([128, 8704], f8, name="wb_sb")
            w1_sb = wb_sb[:, 0:2048].rearrange("p (t k c) -> p t k c", t=4, k=2)
            w2_sb = wb_sb[:, 2048:6656].rearrange(
                "p (t k c) -> p t k c", t=9, k=2
            )
            w3_sb = wb_sb[:, 6656:8704].rearrange("p (k c) -> p k c", k=2)

            bnp_sb = cpool.tile([128, 24], fp32, name="bnp_sb")
            nc.sync.dma_start(bnp_sb, bnp[:])
            sc1_sb = bnp_sb[:, 0:2]
            sh1_sb = bnp_sb[:, 2:4]
            sc2_sb = bnp_sb[:, 4:6]
            sh2_sb = bnp_sb[:, 6:8]
            sc3_sb = bnp_sb[:, 8:16]
            sh3_sb = bnp_sb[:, 16:24]

            # Observer ops: several ISA structs (TensorScalarPtr, Activation
            # with AP scale/bias) only fit ONE sync-wait command, so make
            # each compute engine observe the const DMAs once up front;
            # Tile's vector clock then subsumes those waits downstream.
            scr_a = cpool.tile([128, 24], fp32, name="scr_a")
            nc.scalar.activation(scr_a, bnp_sb, COPY)
            scr_v = cpool.tile([128, 24], fp32, name="scr_v")
            nc.vector.tensor_tensor(scr_v, bnp_sb, bnp_sb, MULT)
            nc.tensor.ldweights(wb_sb[:, 0:128])

            # persistent zero-padded conv2-input buffers (border stays 0;
            # only the 14x14 interior is rewritten each group)
            xb2_bufs = []
            for i in range(2):
                xb2_buf = cpool.tile([128, 2, G, 256], f8, name=f"xb2_{i}")
                nc.scalar.memzero(xb2_buf)
                xb2_bufs.append(xb2_buf)

            # ---- startup: load + binarize ALL inputs up front ------------
            # (ACT's queue is in-order; hoisting the sign-ins keeps later
            # group boundaries from stalling PE behind them. DMA issue
            # order prioritizes what the first matmuls need.)
            xins, xb1s = [], []
            for g in range(NGRP):
                xin = xin_pool.tile([128, 8, G, NPX], fp32, name=f"xin{g}", tag="xin")
                xins.append(xin)
                xb1 = xb1_pool.tile([128, 8, G, NPX], f8, name=f"xb1{g}", tag="xb1")
                xb1s.append(xb1)
            # first group in two halves so sign-in overlaps its own DMA.
            # Only groups 0/1 are sign-binarized up front: ACT's queue is
            # in-order, so sign-in(g) for later groups is emitted inside
            # group g-1 (1-group lookahead) to avoid head-of-line blocking.
            nc.sync.dma_start(wb_sb[:, 0:2048], wb[:, 0:2048])      # w1 first
            for q in range(4):
                nc.sync.dma_start(
                    xins[0][:, 2 * q:2 * q + 2], xt[0, :, 2 * q:2 * q + 2]
                )
            for q in range(4):
                nc.scalar.activation(
                    xb1s[0][:, 2 * q:2 * q + 2], xins[0][:, 2 * q:2 * q + 2],
                    SIGN,
                )
            nc.sync.dma_start(wb_sb[:, 2048:8704], wb[:, 2048:8704])
            for g in range(1, NGRP):
                nc.sync.dma_start(xins[g], xt[g])
            nc.scalar.activation(xb1s[1], xins[1], SIGN)

            for g in range(NGRP):
                xin = xins[g]
                xb1 = xb1s[g]
                xtch = tmp_pool.tile([128, G, 1], fp32, name="xtch", tag="xtch")
                nc.vector.tensor_tensor(
                    xtch, xin[:, 0, :, 0:1], xin[:, 0, :, 0:1], MULT
                )
                # xb2: conv2 input in zero-padded 16x16 spatial layout
                xb2 = xb2_bufs[g % 2]
                for m in range(2):
                    ps1 = ps1_pool.tile([128, G * NPX], fp32, name="ps1")
                    for t in range(4):
                        nc.tensor.matmul(
                            ps1,
                            w1_sb[:, t, :, m * 128:(m + 1) * 128],
                            xb1[:, 2 * t:2 * t + 2].rearrange(
                                "p k b n -> p k (b n)"
                            ),
                            start=(t == 0),
                            stop=(t == 3),
                            perf_mode=DR,
                        )
                    # BN1+sign, bit-exact two-rounding on ACT:
                    #   t = RN(psum*scale)  (Copy w/ per-partition scale)
                    #   xb2 = Sign(fma(t, 1.0, shift)) = Sign(RN(t+shift))
                    t1 = tmp_pool.tile([128, G, NPX], fp32, name="t1", tag="tmp12")
                    nc.scalar.activation(
                        t1,
                        ps1.rearrange("p (b n) -> p b n", b=G),
                        COPY,
                        scale=sc1_sb[:, m:m + 1],
                    )
                    dst = xb2[:, m].rearrange("p b (h w) -> p b h w", h=16)[
                        :, :, 1:15, 1:15
                    ]
                    nc.scalar.activation(
                        dst,
                        t1.rearrange("p b (h w) -> p b h w", h=14),
                        SIGN,
                        bias=sh1_sb[:, m:m + 1],
                    )

                # sign-in for group g+2 (lookahead; ACT is idle-ish here)
                if g + 2 < NGRP:
                    nc.scalar.activation(xb1s[g + 2], xins[g + 2], SIGN)

                # ---- conv2 (3x3 pad 1, 256->256) + BN2 + sign ------------
                xb3 = xb3_pool.tile([128, 2, G, NPX], f8, name="xb3")
                for m in range(2):
                    ps2 = ps2_pool.tile([128, G, 512], fp32, name="ps2")
                    for tap in range(9):
                        ky, kx = tap // 3, tap % 3
                        wsl = w2_sb[:, tap, :, m * 128:(m + 1) * 128]
                        for b in range(G):
                            xv = xb2[:, :, b].rearrange(
                                "p k (h w) -> p k h w", h=16
                            )
                            nc.tensor.matmul(
                                ps2[:, b, :NPX],
                                wsl,
                                xv[:, :, ky:ky + 14, kx:kx + 14],
                                start=(tap == 0),
                                stop=(tap == 8),
                                perf_mode=DR,
                                skip_group_check=True,
                            )
                    t2 = tmp_pool.tile([128, G, NPX], fp32, name="t2", tag="tmp12")
                    nc.scalar.activation(
                        t2,
                        ps2[:, :, :NPX],
                        COPY,
                        scale=sc2_sb[:, m:m + 1],
                    )
                    nc.scalar.activation(
                        xb3[:, m], t2, SIGN, bias=sh2_sb[:, m:m + 1]
                    )

                # ---- conv3 (1x1, 256->1024) + BN3 + residual -------------
                # m-tiles paired into one 2-bank psum tile so each DVE evac
                # op covers 784 elements (amortizes per-op overhead).
                out_sb = out_pool.tile([128, 8, G, NPX], fp32, name="out_sb")
                for mm in range(4):
                    ps3s = []
                    for j in range(2):
                        m = 2 * mm + j
                        ps3 = ps3_pool.tile([128, G * NPX], fp32, name="ps3",
                                            tag="ps3")
                        ps3s.append(ps3)
                        nc.tensor.matmul(
                            ps3,
                            w3_sb[:, :, m * 128:(m + 1) * 128],
                            xb3.rearrange("p k b n -> p k (b n)"),
                            start=True,
                            stop=True,
                            perf_mode=DR,
                        )
                    # BN3 (x*scale+shift, two roundings) + residual add.
                    # (TensorScalarPtr is fine now: Bacc's compile() spills
                    # excess sem waits onto EventSemaphore instructions.)
                    m0 = 2 * mm
                    t3 = tmp_pool.tile([128, 2, G, NPX], fp32, name="t3",
                                       tag="t3")
                    for j in range(2):
                        nc.vector.tensor_scalar(
                            t3[:, j],
                            ps3s[j].rearrange("p (b n) -> p b n", b=G),
                            sc3_sb[:, m0 + j:m0 + j + 1],
                            sh3_sb[:, m0 + j:m0 + j + 1],
                            MULT,
                            ADD,
                        )
                    eng = nc.vector if mm % 2 == 0 else nc.gpsimd
                    eng.tensor_add(
                        out_sb[:, m0:m0 + 2], t3, xin[:, m0:m0 + 2]
                    )
                    nc.sync.dma_start(
                        yt[g, :, m0:m0 + 2], out_sb[:, m0:m0 + 2]
                    )

    nc.compile()
    return nc


def _bn_params(g, b, m, v):
    """scale/shift computed with the same jax expressions as the reference."""
    import jax
    import jax.numpy as jnp
    from jax import lax

    ge, be, me, ve = (jnp.asarray(t) for t in (g, b, m, v))
    scale = ge * lax.rsqrt(ve + _EPS)
    shift = be - ge * me * lax.rsqrt(ve + _EPS)
    return np.asarray(scale, np.float32), np.asarray(shift, np.float32)


def _prep_inputs(inputs):
    """Host-side prep: shard batch, binarize weights, fold BN params."""
    f8 = ml_dtypes.float8_e4m3
    x = np.ascontiguousarray(np.asarray(inputs["x"], np.float32))

    # weights -> sign -> fp8e4 (exact for +-1), DoubleRow-interleaved
    # layouts: [128 ki, kpair, ko, cout] where channel = (2*t+ko)*128+ki
    w1 = np.sign(np.asarray(inputs["w1"], np.float32)[:, :, 0, 0])        # [256,1024]
    w1b = np.ascontiguousarray(
        w1.T.reshape(4, 2, 128, 256).transpose(2, 0, 1, 3).astype(f8)
    )                                                                      # [128,4,2,256]
    w2 = np.sign(np.asarray(inputs["w2"], np.float32))                     # [256,256,3,3]
    w2b = np.ascontiguousarray(
        w2.transpose(1, 2, 3, 0)                                           # [ci,ky,kx,co]
        .reshape(2, 128, 9, 256)                                           # [ko,ki,tap,co]
        .transpose(1, 2, 0, 3)
        .astype(f8)
    )                                                                      # [128,9,2,256]
    w3 = np.sign(np.asarray(inputs["w3"], np.float32)[:, :, 0, 0])         # [1024,256]
    w3b = np.ascontiguousarray(
        w3.T.reshape(2, 128, 1024).transpose(1, 0, 2).astype(f8)
    )                                                                      # [128,2,1024]

    sc1, sh1 = _bn_params(inputs["g1"], inputs["b1"], inputs["m1"], inputs["v1"])
    sc2, sh2 = _bn_params(inputs["g2"], inputs["b2"], inputs["m2"], inputs["v2"])
    sc3, sh3 = _bn_params(inputs["g3"], inputs["b3"], inputs["m3"], inputs["v3"])

    wb = np.concatenate(
        [w1b.reshape(128, -1), w2b.reshape(128, -1), w3b.reshape(128, -1)],
        axis=1,
    )
    bnp = np.concatenate(
        [
            sc1.reshape(2, 128).T, sh1.reshape(2, 128).T,
            sc2.reshape(2, 128).T, sh2.reshape(2, 128).T,
            sc3.reshape(8, 128).T, sh3.reshape(8, 128).T,
        ],
        axis=1,
    ).astype(np.float32)
    common = {
        "wb": np.ascontiguousarray(wb),
        "bnp": np.ascontiguousarray(bnp),
    }

    # x -> per-core [NGRP, 128, 8kt, G, 196]
    xr = x.reshape(N_CORES, NGRP, G, 8, 128, NPX)  # (core, grp, img, kt, p, n)
    in_maps = []
    for c in range(N_CORES):
        xt = np.ascontiguousarray(xr[c].transpose(0, 3, 2, 1, 4))
        in_maps.append({"xt": xt, **common})
    return in_maps


def _assemble_output(results):
    """results: list of per-core dicts with 'yt' [NGRP,128,8,G,196]."""
    y = np.empty((N_CORES, NGRP, G, 8, 128, NPX), np.float32)
    for c, r in enumerate(results):
        y[c] = np.asarray(r["yt"]).transpose(0, 3, 2, 1, 4)
    return np.ascontiguousarray(
        y.reshape(B, CIN, 14, 14)
    )


def _run(inputs, trace=False):
    from concourse.bass_utils import run_bass_kernel_spmd

    if "nc" not in _state:
        _state["nc"] = _build_nc()
    nc = _state["nc"]
    in_maps = _prep_inputs(inputs)
    res = run_bass_kernel_spmd(
        nc, in_maps, core_ids=list(range(N_CORES)), trace=trace
    )
    return _assemble_output(res.results), res


def kernel(**inputs):
    out, _ = _run(inputs, trace=False)
    return out


# revision 30
# speedup vs baseline: 1.1408x; 1.0667x over previous
"""Binarized ResNet Bottleneck block (sign-binarized convs + BN + residual)
for Trainium2, data-parallel over 8 NeuronCores (8 images per core).

Math (per reference):
  out1 = BN1(conv1x1(sign(x),  sign(w1)))        # 1024 -> 256
  out2 = BN2(conv3x3(sign(out1), sign(w2)))      # 256 -> 256, pad 1
  out3 = BN3(conv1x1(sign(out2), sign(w3)))      # 256 -> 1024
  y    = out3 + x
(The htanh's in the reference only feed sign(), and sign(htanh(t)) == sign(t),
so they are dropped. Binarized values are exactly +-1 (or 0) in bf16 and conv
accumulations are exact small integers in fp32 PSUM, so matmuls are exact.)

Layout strategy per core (8 images, processed in 4 groups of G=2):
  - activations live in SBUF as [128 chan-partitions, chan_tile, img, pixels]
  - conv = accumulated 128x128xN matmuls; conv2's 3x3 uses 9 shifted-window
    matmuls over a zero-padded 16x16 per-image spatial layout.
  - BN (x*scale+shift, separate f32 roundings on VectorE to match the
    reference's unfused mul+add) then Sign on ScalarE between layers.
  - residual add on VectorE, output DMA'd back in a host-friendly layout.
"""

import os
import sys

import numpy as np
import ml_dtypes

N_CORES = 8
B = 64              # global batch
CIN = 1024
P = 256             # bottleneck width
NPX = 196           # 14*14
G = 2               # images per group
NGRP = 4            # groups per core  (8 images / G)
BPC = B // N_CORES  # images per core

_EPS = 1e-5

_state = {}


def _build_nc():
    import concourse.bass as bass
    import concourse.mybir as mybir
    from concourse import bacc
    from concourse.tile import TileContext

    fp32 = mybir.dt.float32
    bf16 = mybir.dt.bfloat16
    f8 = mybir.dt.float8e4
    DR = mybir.MatmulPerfMode.DoubleRow
    SIGN = mybir.ActivationFunctionType.Sign
    COPY = mybir.ActivationFunctionType.Copy
    IDENT = mybir.ActivationFunctionType.Identity
    MULT = mybir.AluOpType.mult
    ADD = mybir.AluOpType.add

    # Bacc (not plain Bass): its compile() pass splits multi-sem waits into
    # EventSemaphore instructions (HW allows only 1 wait per instruction).
    nc = bacc.Bacc(None, target_bir_lowering=False)

    xt = nc.dram_tensor("xt", [NGRP, 128, 8, G, NPX], fp32, kind="ExternalInput")
    # all binarized fp8 weights in one tensor, DoubleRow-interleaved:
    # cols [0:2048]=w1 (4 kpair x 2 ko x 256), [2048:6656]=w2 (9 tap x 2 ko
    # x 256), [6656:8704]=w3 (2 ko x 1024)
    wb = nc.dram_tensor("wb", [128, 8704], f8, kind="ExternalInput")
    # BN params in one tensor: sc1(2) sh1(2) sc2(2) sh2(2) sc3(8) sh3(8)
    bnp = nc.dram_tensor("bnp", [128, 24], fp32, kind="ExternalInput")
    yt = nc.dram_tensor("yt", [NGRP, 128, 8, G, NPX], fp32, kind="ExternalOutput")

    with TileContext(nc) as tc:
        with (
            tc.tile_pool(name="consts", bufs=1) as cpool,
            tc.tile_pool(name="xin_pool", bufs=4) as xin_pool,
            tc.tile_pool(name="xb1_pool", bufs=4) as xb1_pool,
            tc.tile_pool(name="xb2_pool", bufs=2) as xb2_pool,
            tc.tile_pool(name="xb3_pool", bufs=2) as xb3_pool,
            tc.tile_pool(name="tmp_pool", bufs=4) as tmp_pool,
            tc.tile_pool(name="out_pool", bufs=2) as out_pool,
            tc.tile_pool(name="ps1_pool", bufs=2, space="PSUM") as ps1_pool,
            tc.tile_pool(name="ps2_pool", bufs=2, space="PSUM") as ps2_pool,
            tc.tile_pool(name="ps3_pool", bufs=2, space="PSUM") as ps3_pool,
        ):
            wb_sb = cpool.tile([128, 8704], f8, name="wb_sb")
            w1_sb = wb_sb[:, 0:2048].rearrange("p (t k c) -> p t k c", t=4, k=2)
            w2_sb = wb_sb[:, 2048:6656].rearrange(
                "p (t k c) -> p t k c", t=9, k=2
            )
            w3_sb = wb_sb[:, 6656:8704].rearrange("p (k c) -> p k c", k=2)

            bnp_sb = cpool.tile([128, 24], fp32, name="bnp_sb")
            nc.sync.dma_start(bnp_sb, bnp[:])
            sc1_sb = bnp_sb[:, 0:2]
            sh1_sb = bnp_sb[:, 2:4]
            sc2_sb = bnp_sb[:, 4:6]
            sh2_sb = bnp_sb[:, 6:8]
            sc3_sb = bnp_sb[:, 8:16]
            sh3_sb = bnp_sb[:, 16:24]

            # Observer ops: several ISA structs (TensorScalarPtr, Activation
            # with AP scale/bias) only fit ONE sync-wait command, so make
            # each compute engine observe the const DMAs once up front;
            # Tile's vector clock then subsumes those waits downstream.
            scr_a = cpool.tile([128, 24], fp32, name="scr_a")
            nc.scalar.activation(scr_a, bnp_sb, COPY)
            scr_v = cpool.tile([128, 24], fp32, name="scr_v")
            nc.vector.tensor_tensor(scr_v, bnp_sb, bnp_sb, MULT)
            nc.tensor.ldweights(wb_sb[:, 0:128])

            # persistent zero-padded conv2-input buffers (border stays 0;
            # only the 14x14 interior is rewritten each group)
            xb2_bufs = []
            for i in range(2):
                xb2_buf = cpool.tile([128, 2, G, 256], f8, name=f"xb2_{i}")
                nc.scalar.memzero(xb2_buf)
                xb2_bufs.append(xb2_buf)

            # ---- startup: load + binarize ALL inputs up front ------------
            # (ACT's queue is in-order; hoisting the sign-ins keeps later
            # group boundaries from stalling PE behind them. DMA issue
            # order prioritizes what the first matmuls need.)
            xins, xb1s = [], []
            for g in range(NGRP):
                xin = xin_pool.tile([128, 8, G, NPX], fp32, name=f"xin{g}", tag="xin")
                xins.append(xin)
                xb1 = xb1_pool.tile([128, 8, G, NPX], f8, name=f"xb1{g}", tag="xb1")
                xb1s.append(xb1)
            # first group in two halves so sign-in overlaps its own DMA.
            # Only groups 0/1 are sign-binarized up front: ACT's queue is
            # in-order, so sign-in(g) for later groups is emitted inside
            # group g-1 (1-group lookahead) to avoid head-of-line blocking.
            nc.sync.dma_start(wb_sb[:, 0:2048], wb[:, 0:2048])      # w1 first
            for q in range(4):
                nc.sync.dma_start(
                    xins[0][:, 2 * q:2 * q + 2], xt[0, :, 2 * q:2 * q + 2]
                )
            for q in range(4):
                nc.scalar.activation(
                    xb1s[0][:, 2 * q:2 * q + 2], xins[0][:, 2 * q:2 * q + 2],
                    SIGN,
                )
            nc.sync.dma_start(wb_sb[:, 2048:8704], wb[:, 2048:8704])
            for g in range(1, NGRP):
                nc.sync.dma_start(xins[g], xt[g])
            nc.scalar.activation(xb1s[1], xins[1], SIGN)

            for g in range(NGRP):
                xin = xins[g]
                xb1 = xb1s[g]
                xtch = tmp_pool.tile([128, G, 1], fp32, name="xtch", tag="xtch")
                nc.vector.tensor_tensor(
                    xtch, xin[:, 0, :, 0:1], xin[:, 0, :, 0:1], MULT
                )
                # xb2: conv2 input in zero-padded 16x16 spatial layout
                xb2 = xb2_bufs[g % 2]
                for m in range(2):
                    ps1 = ps1_pool.tile([128, G * NPX], fp32, name="ps1")
                    for t in range(4):
                        nc.tensor.matmul(
                            ps1,
                            w1_sb[:, t, :, m * 128:(m + 1) * 128],
                            xb1[:, 2 * t:2 * t + 2].rearrange(
                                "p k b n -> p k (b n)"
                            ),
                            start=(t == 0),
                            stop=(t == 3),
                            perf_mode=DR,
                        )
                    # BN1+sign in one ACT op (fma(psum, scale, shift) -> Sign)
                    dst = xb2[:, m].rearrange("p b (h w) -> p b h w", h=16)[
                        :, :, 1:15, 1:15
                    ]
                    nc.scalar.activation(
                        dst,
                        ps1.rearrange("p (b h w) -> p b h w", b=G, h=14),
                        SIGN,
                        bias=sh1_sb[:, m:m + 1],
                        scale=sc1_sb[:, m:m + 1],
                    )

                # sign-in for group g+2 (lookahead; ACT is idle-ish here)
                if g + 2 < NGRP:
                    nc.scalar.activation(xb1s[g + 2], xins[g + 2], SIGN)

                # ---- conv2 (3x3 pad 1, 256->256) + BN2 + sign ------------
                xb3 = xb3_pool.tile([128, 2, G, NPX], f8, name="xb3")
                for m in range(2):
                    ps2 = ps2_pool.tile([128, G, 512], fp32, name="ps2")
                    for tap in range(9):
                        ky, kx = tap // 3, tap % 3
                        wsl = w2_sb[:, tap, :, m * 128:(m + 1) * 128]
                        for b in range(G):
                            xv = xb2[:, :, b].rearrange(
                                "p k (h w) -> p k h w", h=16
                            )
                            nc.tensor.matmul(
                                ps2[:, b, :NPX],
                                wsl,
                                xv[:, :, ky:ky + 14, kx:kx + 14],
                                start=(tap == 0),
                                stop=(tap == 8),
                                perf_mode=DR,
                                skip_group_check=True,
                            )
                    # BN2+sign in one ACT op (fma(psum, scale, shift) -> Sign)
                    nc.scalar.activation(
                        xb3[:, m],
                        ps2[:, :, :NPX],
                        SIGN,
                        bias=sh2_sb[:, m:m + 1],
                        scale=sc2_sb[:, m:m + 1],
                    )

                # ---- conv3 (1x1, 256->1024) + BN3 + residual -------------
                # m-tiles paired into one 2-bank psum tile so each DVE evac
                # op covers 784 elements (amortizes per-op overhead).
                out_sb = out_pool.tile([128, 8, G, NPX], fp32, name="out_sb")
                for mm in range(4):
                    ps3s = []
                    for j in range(2):
                        m = 2 * mm + j
                        ps3 = ps3_pool.tile([128, G * NPX], fp32, name="ps3",
                                            tag="ps3")
                        ps3s.append(ps3)
                        nc.tensor.matmul(
                            ps3,
                            w3_sb[:, :, m * 128:(m + 1) * 128],
                            xb3.rearrange("p k b n -> p k (b n)"),
                            start=True,
                            stop=True,
                            perf_mode=DR,
                        )
                    # BN3 (x*scale+shift, two roundings) + residual add.
                    # (TensorScalarPtr is fine now: Bacc's compile() spills
                    # excess sem waits onto EventSemaphore instructions.)
                    m0 = 2 * mm
                    t3 = tmp_pool.tile([128, 2, G, NPX], fp32, name="t3",
                                       tag="t3")
                    for j in range(2):
                        if j == 0:
                            nc.scalar.activation(
                                t3[:, j],
                                ps3s[j].rearrange("p (b n) -> p b n", b=G),
                                IDENT,
                                bias=sh3_sb[:, m0 + j:m0 + j + 1],
                                scale=sc3_sb[:, m0 + j:m0 + j + 1],
                            )
                        else:
                            nc.vector.tensor_scalar(
                                t3[:, j],
                                ps3s[j].rearrange("p (b n) -> p b n", b=G),
                                sc3_sb[:, m0 + j:m0 + j + 1],
                                sh3_sb[:, m0 + j:m0 + j + 1],
                                MULT,
                                ADD,
                            )
                    nc.vector.tensor_add(
                        out_sb[:, m0:m0 + 2], t3, xin[:, m0:m0 + 2]
                    )
                    nc.sync.dma_start(
                        yt[g, :, m0:m0 + 2], out_sb[:, m0:m0 + 2]
                    )

    nc.compile()
    return nc


def _bn_params(g, b, m, v):
    """scale/shift computed with the same jax expressions as the reference."""
    import jax
    import jax.numpy as jnp
    from jax import lax

    ge, be, me, ve = (jnp.asarray(t) for t in (g, b, m, v))
    scale = ge * lax.rsqrt(ve + _EPS)
    shift = be - ge * me * lax.rsqrt(ve + _EPS)
    return np.asarray(scale, np.float32), np.asarray(shift, np.float32)


def _prep_inputs(inputs):
    """Host-side prep: shard batch, binarize weights, fold BN params."""
    f8 = ml_dtypes.float8_e4m3
    x = np.ascontiguousarray(np.asarray(inputs["x"], np.float32))

    # weights -> sign -> fp8e4 (exact for +-1), DoubleRow-interleaved
    # layouts: [128 ki, kpair, ko, cout] where channel = (2*t+ko)*128+ki
    w1 = np.sign(np.asarray(inputs["w1"], np.float32)[:, :, 0, 0])        # [256,1024]
    w1b = np.ascontiguousarray(
        w1.T.reshape(4, 2, 128, 256).transpose(2, 0, 1, 3).astype(f8)
    )                                                                      # [128,4,2,256]
    w2 = np.sign(np.asarray(inputs["w2"], np.float32))                     # [256,256,3,3]
    w2b = np.ascontiguousarray(
        w2.transpose(1, 2, 3, 0)                                           # [ci,ky,kx,co]
        .reshape(2, 128, 9, 256)                                           # [ko,ki,tap,co]
        .transpose(1, 2, 0, 3)
        .astype(f8)
    )                                                                      # [128,9,2,256]
    w3 = np.sign(np.asarray(inputs["w3"], np.float32)[:, :, 0, 0])         # [1024,256]
    w3b = np.ascontiguousarray(
        w3.T.reshape(2, 128, 1024).transpose(1, 0, 2).astype(f8)
    )                                                                      # [128,2,1024]

    sc1, sh1 = _bn_params(inputs["g1"], inputs["b1"], inputs["m1"], inputs["v1"])
    sc2, sh2 = _bn_params(inputs["g2"], inputs["b2"], inputs["m2"], inputs["v2"])
    sc3, sh3 = _bn_params(inputs["g3"], inputs["b3"], inputs["m3"], inputs["v3"])

    wb = np.concatenate(
        [w1b.reshape(128, -1), w2b.reshape(128, -1), w3b.reshape(128, -1)],
        axis=1,
    )
    bnp = np.concatenate(
        [
            sc1.reshape(2, 128).T, sh1.reshape(2, 128).T,
            sc2.reshape(2, 128).T, sh2.reshape(2, 128).T,
            sc3.reshape(8, 128).T, sh3.reshape(8, 128).T,
        ],
        axis=1,
    ).astype(np.float32)
    common = {
        "wb": np.ascontiguousarray(wb),
        "bnp": np.ascontiguousarray(bnp),
    }

    # x -> per-core [NGRP, 128, 8kt, G, 196]
    xr = x.reshape(N_CORES, NGRP, G, 8, 128, NPX)  # (core, grp, img, kt, p, n)
    in_maps = []
    for c in range(N_CORES):
        xt = np.ascontiguousarray(xr[c].transpose(0, 3, 2, 1, 4))
        in_maps.append({"xt": xt, **common})
    return in_maps


def _assemble_output(results):
    """results: list of per-core dicts with 'yt' [NGRP,128,8,G,196]."""
    y = np.empty((N_CORES, NGRP, G, 8, 128, NPX), np.float32)
    for c, r in enumerate(results):
        y[c] = np.asarray(r["yt"]).transpose(0, 3, 2, 1, 4)
    return np.ascontiguousarray(
        y.reshape(B, CIN, 14, 14)
    )


def _run(inputs, trace=False):
    from concourse.bass_utils import run_bass_kernel_spmd

    if "nc" not in _state:
        _state["nc"] = _build_nc()
    nc = _state["nc"]
    in_maps = _prep_inputs(inputs)
    res = run_bass_kernel_spmd(
        nc, in_maps, core_ids=list(range(N_CORES)), trace=trace
    )
    return _assemble_output(res.results), res


def kernel(**inputs):
    out, _ = _run(inputs, trace=False)
    return out
